# revision 1
# baseline (speedup 1.0000x reference)
"""Trainium2 Bass kernel for Transformer-XL style relative-position MHA.

Problem shapes (hardcoded): B=8, SEG=512, MEM=512, MODEL=1024, H=16, D=64.
Sharding: pure data-parallel over batch -> core b computes batch element b.

IMPORTANT quirk: the reference splits heads with a FLAT reshape
(torch .view), so head h's queries are Q[32h:32h+32, :].reshape(512, 64)
(rows are row-blocks of the projection output; a "query index" i maps to
row 32h + i//16, columns 64*(i%16):+64). Same for k/v/r with 64-row
blocks of [TOT, HD] outputs. The per-head matrices therefore need a
relayout, done via small DRAM round trips with strided access patterns
(DVE cannot cross partitions). u1/u2 additions are folded into the score
matmuls as a 65th contraction row (lhs row = ones, rhs row = u.k / u.r).

Math per core/head in the reinterpreted [S=512] x [T=1024] index space:
  ac[i,j] = (q_i+u1).k_j ; bd in diagonal coords m = j-i+511 (plain matmul)
  p = exp(ac/8)*exp(bd/8) (causal mask j<=i+512), normalized; out = p@v
  y = LN(att @ mlp_w + x)*gamma + beta

The circulant shift is applied by writing EB=exp(bd/8) [S,H,T] bf16 to
DRAM and reading it back with a skewed access pattern (row step H*T-1
elements). Out-of-range m spills into the next head's row; those
positions are exactly the causally-masked ones and are zeroed by a
triangular band mask on the last 128 columns of each 128-row tile.

Projections run as float32r matmuls (full PE rate); scores/pv/mlp in bf16.
"""

import functools
import sys

import numpy as np

sys.path.insert(0, "/opt/trn_rl_repo")

import ml_dtypes  # noqa: E402

import concourse.bass as bass  # noqa: E402
import concourse.mybir as mybir  # noqa: E402
import concourse.tile as tile  # noqa: E402

B, SEG, MEM, MODEL, H, D = 8, 512, 512, 1024, 16, 64
TOT = SEG + MEM
HD = H * D
NCORES = 8
IT = SEG // 128                # 4 row tiles of 128 queries
JMAX = [640, 768, 896, 1024]   # per row-tile: columns beyond are fully masked
MMIN = [384, 256, 128, 0]      # per row-tile: smallest rel index m read

F32 = mybir.dt.float32
F32R = mybir.dt.float32r
BF16 = mybir.dt.bfloat16
AF = mybir.ActivationFunctionType
OP = mybir.AluOpType

bf16_np = ml_dtypes.bfloat16


def _r(ap):
    return ap.bitcast(F32R)


def _emit(tc, t):
    nc = tc.nc
    ctxs = []

    def pool(name, bufs, space="SBUF"):
        p = tc.tile_pool(name=name, bufs=bufs, space=space)
        ctxs.append(p)
        return p.__enter__()

    csts = pool("csts", 1)
    wpool = pool("wpool", 8)      # streaming weights [128,1024] f32 / bf16
    bigp = pool("bigp", 8)        # hT -> rT-input -> x rotation [128,1024] f32
    srcp = pool("srcp", 4)        # projection bf16 staging [128,1024]
    layp = pool("layp", 1)        # qlay/klay-or-rlay [65, *] bf16
    midp = pool("midp", 1)        # vlayT group / attP (shared lifetime)
    vlp = pool("vlp", 8)          # vlay [128, 16*64] bf16 x 8 j-tiles
    athp = pool("athp", 1)        # attTh [64, 8192] bf16
    attp = pool("attp", 8)        # attT [128,512] bf16
    work = pool("work", 2)
    small = pool("small", 3)
    ps = pool("ps", 2, space="PSUM")      # [128,1024] f32 (2 banks)
    pst = pool("pst", 2, space="PSUM")    # transpose out [128,128] bf16 / [1,512]
    psv = pool("psv", 2, space="PSUM")    # pv accum [64,128] f32

    # ---- constants ----
    ident = csts.tile([128, 128], BF16, tag="ident")
    nc.sync.dma_start(ident, t["idm"][:, :])
    tri = csts.tile([128, 128], BF16, tag="tri")
    nc.sync.dma_start(tri, t["trim"][:, :])
    u1l = csts.tile([64, 16], BF16, tag="u1l")
    nc.sync.dma_start(u1l, t["u1l"][:, :])
    u2l = csts.tile([64, 16], BF16, tag="u2l")
    nc.sync.dma_start(u2l, t["u2l"][:, :])
    masks = csts.tile([128, 4], F32, tag="masks")
    nc.sync.dma_start(masks, t["maskc"][:, :])
    epsb = csts.tile([128, 1], F32, tag="epsb")
    nc.vector.memset(epsb, 1e-5)
    gam = csts.tile([128, MODEL], BF16, tag="gam")
    nc.gpsimd.dma_start(gam, bass.AP(tensor=t["gam"], offset=0, ap=[[0, 128], [1, MODEL]]))
    bet = csts.tile([128, MODEL], BF16, tag="bet")
    nc.gpsimd.dma_start(bet, bass.AP(tensor=t["bet"], offset=0, ap=[[0, 128], [1, MODEL]]))

    ebuf = t["ebuf"]

    # ---- zero strip: rows [0,384) x all heads x m in [0,128) of ebuf ----
    zs = csts.tile([128, 8 * 128], BF16, tag="zs")
    nc.vector.memset(zs, 0.0)
    for blk in range(3):
        for hb in range(2):
            dst = bass.AP(
                tensor=ebuf,
                offset=blk * 128 * H * TOT + hb * 8 * TOT,
                ap=[[H * TOT, 128], [TOT, 8], [1, 128]],
            )
            nc.sync.dma_start(dst, zs.rearrange("p (h m) -> p h m", h=8))

    # ---- load hT ----
    hts = []
    for mt in range(8):
        ht = bigp.tile([128, TOT], F32R, tag="big", name=f"ht{mt}")
        nc.sync.dma_start(ht, t["hT"][mt * 128:(mt + 1) * 128, :].bitcast(F32R))
        hts.append(ht)

    def stream_w(key, dtype=F32R):
        ws = []
        for mt in range(8):
            w = wpool.tile([128, HD], dtype, tag="w", name=f"{key}{mt}")
            src = t[key][mt * 128:(mt + 1) * 128, :]
            if dtype == F32R:
                src = src.bitcast(F32R)
            nc.sync.dma_start(w, src)
            ws.append(w)
        return ws

    # ---- projections -> bf16 staging -> DRAM ----
    # QT [HD, SEG] (rhs = xT part of hT); KT/VT/RT [HD, TOT]
    def project(wkey, rhs_tiles, rhs_sl, n, dram, st_cols):
        ws = stream_w(wkey)
        for dt in range(8):
            acc = ps.tile([128, n], F32, tag="mm", name=f"{wkey}mm{dt}")
            for c0 in range(0, n, 512):
                for mt in range(8):
                    nc.tensor.matmul(
                        acc[:, c0:c0 + 512],
                        lhsT=ws[mt][:, dt * 128:(dt + 1) * 128],
                        rhs=rhs_tiles[mt][:, rhs_sl[0] + c0:rhs_sl[0] + c0 + 512],
                        start=(mt == 0),
                        stop=(mt == 7),
                    )
            stg = srcp.tile([128, st_cols], BF16, tag="stg", name=f"{wkey}st{dt}")
            nc.scalar.copy(stg[:, :n], acc)
            nc.sync.dma_start(
                bass.AP(tensor=dram, offset=dt * 128 * n, ap=[[n, 128], [1, n]]),
                stg[:, :n],
            )

    project("wq", hts, (SEG,), SEG, t["qtd"], TOT)
    project("wk", hts, (0,), TOT, t["ktd"], TOT)
    project("wv", hts, (0,), TOT, t["vtd"], TOT)
    rts = []
    for mt in range(8):
        rt_in = bigp.tile([128, TOT], F32R, tag="big", name=f"rtin{mt}")
        nc.sync.dma_start(rt_in, t["rT"][mt * 128:(mt + 1) * 128, :].bitcast(F32R))
        rts.append(rt_in)
    project("wr", rts, (0,), TOT, t["rtd"], TOT)

    # ---- qlay [65, 16*512] bf16: qlay[dd, h*512+16rr+cc] = QT[64cc+dd, 32h+rr]
    qlay = layp.tile([65, H * SEG], BF16, tag="qlay")
    for cc in range(16):
        src = bass.AP(
            tensor=t["qtd"],
            offset=(64 * cc) * SEG,
            ap=[[SEG, 64], [32, 16], [1, 32]],
        )
        dst = bass.AP(
            tensor=qlay[:, :].tensor,
            offset=qlay[:, :].offset + cc,
            ap=[[qlay[:, :].ap[0][0], 64], [512, 16], [16, 32]],
        )
        nc.sync.dma_start(dst, src)
    nc.vector.memset(qlay[64:65, :], 1.0)

    def build_lay(dram, name, urow_src):
        """lay [65, 16*1024]: lay[dd, h*1024+16ss+tt] = SRC[64tt+dd, 64h+ss];
        row 64 per head = urow_src . lay_head (computed via K=64 matmuls)."""
        lay = layp.tile([65, H * TOT], BF16, tag="lay", name=name)
        lstep = lay[:, :].ap[0][0]
        loff = lay[:, :].offset
        for tt in range(16):
            for hf in range(2):
                src = bass.AP(
                    tensor=dram,
                    offset=(64 * tt + 32 * hf) * TOT,
                    ap=[[TOT, 32], [64, 16], [1, 64]],
                )
                dst = bass.AP(
                    tensor=lay[32 * hf:65, :].tensor,
                    offset=loff + 32 * hf * lstep + tt,
                    ap=[[lstep, 32], [1024, 16], [16, 64]],
                )
                nc.sync.dma_start(dst, src)
        for h in range(16):
            for c0 in range(0, TOT, 512):
                up = pst.tile([1, 512], F32, tag="tp", name=f"{name}u{h}_{c0}")
                nc.tensor.matmul(
                    up,
                    lhsT=urow_src[:, h:h + 1],
                    rhs=lay[0:64, h * TOT + c0:h * TOT + c0 + 512],
                    start=True,
                    stop=True,
                )
                nc.scalar.copy(lay[64:65, h * TOT + c0:h * TOT + c0 + 512], up)
        return lay

    # ---- rlay + EB production ----
    rlay = build_lay(t["rtd"], "rlay", u2l)
    for h in range(16):
        for it in range(IT):
            m0, w_ = MMIN[it], TOT - MMIN[it]
            bd = ps.tile([128, w_], F32, tag="mm", name=f"bd{h}_{it}")
            c0 = 0
            while c0 < w_:
                cw = min(512, w_ - c0)
                nc.tensor.matmul(
                    bd[:, c0:c0 + cw],
                    lhsT=qlay[:, h * SEG + it * 128:h * SEG + (it + 1) * 128],
                    rhs=rlay[:, h * TOT + m0 + c0:h * TOT + m0 + c0 + cw],
                    start=True,
                    stop=True,
                )
                c0 += cw
            eb = work.tile([128, TOT], BF16, tag="ebw", name=f"eb{h}_{it}")
            nc.scalar.activation(eb[:, :w_], bd, AF.Exp, scale=0.125)
            dst = bass.AP(
                tensor=ebuf,
                offset=it * 128 * H * TOT + h * TOT + m0,
                ap=[[H * TOT, 128], [1, w_]],
            )
            nc.sync.dma_start(dst, eb[:, :w_])

    # ---- klay (reuses rlay's slot after rlay is dead) ----
    klay = build_lay(t["ktd"], "klay", u1l)

    # ---- vlay: vlayT group [64, 4*1024] from vtd, then PE-transpose ----
    vls = [
        vlp.tile([128, H * 64], BF16, tag="vl", name=f"vl{jb}") for jb in range(8)
    ]
    for g in range(4):
        vtg = midp.tile([64, 4 * TOT], BF16, tag="mid", name=f"vtg{g}")
        vstep = vtg[:, :].ap[0][0]
        voff = vtg[:, :].offset
        for tt in range(16):
            for hf in range(2):
                src = bass.AP(
                    tensor=t["vtd"],
                    offset=(64 * tt + 32 * hf) * TOT + 64 * 4 * g,
                    ap=[[TOT, 32], [64, 4], [1, 64]],
                )
                dst = bass.AP(
                    tensor=vtg[:, :].tensor,
                    offset=voff + 32 * hf * vstep + tt,
                    ap=[[vstep, 32], [1024, 4], [16, 64]],
                )
                nc.sync.dma_start(dst, src)
        for hh in range(4):
            h = 4 * g + hh
            for jb in range(8):
                tp = pst.tile([128, 64], BF16, tag="tp", name=f"vt{h}_{jb}")
                nc.tensor.transpose(
                    tp, vtg[0:64, hh * TOT + jb * 128:hh * TOT + (jb + 1) * 128],
                    ident[0:64, 0:64],
                )
                nc.scalar.copy(vls[jb][:, h * 64:(h + 1) * 64], tp)

    # ---- scores / softmax / p@v ----
    attTh = athp.tile([64, H * SEG], BF16, tag="atth")
    for it in range(IT):
        jm = JMAX[it]
        nblk = jm // 128
        i0 = it * 128
        for h in range(16):
            ebs = work.tile([128, jm], BF16, tag="ebs", name=f"ebs{it}_{h}")
            src = bass.AP(
                tensor=ebuf,
                offset=i0 * H * TOT + h * TOT + (511 - i0),
                ap=[[H * TOT - 1, 128], [1, jm]],
            )
            nc.sync.dma_start(ebs, src)
            acps = ps.tile([128, jm], F32, tag="mm", name=f"ac{it}_{h}")
            c0 = 0
            while c0 < jm:
                cw = min(512, jm - c0)
                nc.tensor.matmul(
                    acps[:, c0:c0 + cw],
                    lhsT=qlay[:, h * SEG + i0:h * SEG + i0 + 128],
                    rhs=klay[:, h * TOT + c0:h * TOT + c0 + cw],
                    start=True,
                    stop=True,
                )
                c0 += cw
            ea = work.tile([128, jm], BF16, tag="ebw", name=f"ea{it}_{h}")
            nc.scalar.activation(ea, acps, AF.Exp, scale=0.125)
            nc.vector.tensor_mul(ebs[:, jm - 128:jm], ebs[:, jm - 128:jm], tri)
            p = work.tile([128, jm], BF16, tag="p", name=f"p{it}_{h}")
            sums = small.tile([128, 1], F32, tag="sums", name=f"sm{it}_{h}")
            nc.vector.scalar_tensor_tensor(
                out=p, in0=ea, scalar=1.0, in1=ebs,
                op0=OP.mult, op1=OP.mult, accum_out=sums,
            )
            rec = small.tile([128, 1], F32, tag="rec", name=f"rc{it}_{h}")
            nc.vector.reciprocal(rec, sums)
            alpha = small.tile([128, 1], F32, tag="alpha", name=f"al{it}_{h}")
            nc.vector.tensor_mul(alpha, rec, masks[:, it:it + 1])
            nc.vector.tensor_scalar_mul(p, p, alpha)
            pts = work.tile([128, jm], BF16, tag="pts", name=f"pt{it}_{h}")
            for jb in range(nblk):
                tp = pst.tile([128, 128], BF16, tag="tp", name=f"tp{it}_{h}_{jb}")
                nc.tensor.transpose(tp, p[:, jb * 128:(jb + 1) * 128], ident)
                nc.scalar.copy(pts[:, jb * 128:(jb + 1) * 128], tp)
            pv = psv.tile([64, 128], F32, tag="pv", name=f"pv{it}_{h}")
            for jb in range(nblk):
                nc.tensor.matmul(
                    pv,
                    lhsT=vls[jb][:, 64 * h:64 * h + 64],
                    rhs=pts[:, jb * 128:(jb + 1) * 128],
                    start=(jb == 0),
                    stop=(jb == nblk - 1),
                )
            nc.scalar.copy(attTh[:, h * SEG + i0:h * SEG + i0 + 128], pv)

    # ---- att DRAM hop: attP[dd, cc*512+32h+rr] = attTh[dd, h*512+16rr+cc] ----
    attP = midp.tile([64, H * SEG], BF16, tag="mid", name="attP")
    ao = attTh[:, :].offset
    astep = attTh[:, :].ap[0][0]
    src = bass.AP(
        tensor=attTh[:, :].tensor, offset=ao,
        ap=[[astep, 64], [1, 16], [512, 16], [16, 32]],
    )
    po = attP[:, :].offset
    pstep = attP[:, :].ap[0][0]
    dst = bass.AP(
        tensor=attP[:, :].tensor, offset=po,
        ap=[[pstep, 64], [512, 16], [32, 16], [1, 32]],
    )
    nc.vector.tensor_copy(dst, src)
    nc.sync.dma_start(
        bass.AP(tensor=t["attd"], offset=0, ap=[[H * SEG, 64], [1, H * SEG]]),
        attP,
    )
    atts = []
    for a in range(8):
        at = attp.tile([128, SEG], BF16, tag="att", name=f"att{a}")
        for ccp in range(2):
            src = bass.AP(
                tensor=t["attd"],
                offset=(2 * a + ccp) * 512,
                ap=[[H * SEG, 64], [1, 512]],
            )
            nc.sync.dma_start(at[ccp * 64:(ccp + 1) * 64, :], src)
        atts.append(at)

    # ---- mlp + residual + layernorm ----
    mlps = stream_w("mlpw", BF16)
    xs = []
    for it in range(IT):
        x = bigp.tile([128, MODEL], F32, tag="big", name=f"x{it}")
        nc.sync.dma_start(x, t["x_sm"][it * 128:(it + 1) * 128, :])
        xs.append(x)
    for it in range(IT):
        acc = ps.tile([128, MODEL], F32, tag="mm", name=f"mlp{it}")
        for half in range(2):
            for dt in range(8):
                nc.tensor.matmul(
                    acc[:, half * 512:(half + 1) * 512],
                    lhsT=atts[dt][:, it * 128:(it + 1) * 128],
                    rhs=mlps[dt][:, half * 512:(half + 1) * 512],
                    start=(dt == 0),
                    stop=(dt == 7),
                )
        y = work.tile([128, MODEL], F32, tag="y", name=f"y{it}", bufs=1)
        ysum = small.tile([128, 1], F32, tag="ysum", name=f"ys{it}")
        nc.vector.scalar_tensor_tensor(
            out=y, in0=acc, scalar=1.0, in1=xs[it],
            op0=OP.mult, op1=OP.add, accum_out=ysum,
        )
        sq = ps.tile([128, MODEL], F32, tag="mm", name=f"sq{it}")
        ysq = small.tile([128, 1], F32, tag="ysq", name=f"yq{it}")
        nc.scalar.activation(sq, y, AF.Square, accum_out=ysq)
        mu = small.tile([128, 1], F32, tag="mu", name=f"mu{it}")
        nc.scalar.mul(mu, ysum, 1.0 / MODEL)
        msq = small.tile([128, 1], F32, tag="msq", name=f"mq{it}")
        nc.scalar.mul(msq, ysq, 1.0 / MODEL)
        mu2 = small.tile([128, 1], F32, tag="mu2", name=f"m2{it}")
        nc.vector.tensor_mul(mu2, mu, mu)
        var = small.tile([128, 1], F32, tag="var", name=f"va{it}")
        nc.vector.tensor_tensor(out=var, in0=msq, in1=mu2, op=OP.subtract)
        std = small.tile([128, 1], F32, tag="std", name=f"sd{it}")
        nc.scalar.activation(std, var, AF.Sqrt, bias=epsb)
        rstd = small.tile([128, 1], F32, tag="rstd", name=f"rs{it}")
        nc.vector.reciprocal(rstd, std)
        o = work.tile([128, MODEL], F32, tag="o", name=f"o{it}", bufs=1)
        nc.vector.tensor_scalar(
            out=o, in0=y, scalar1=mu, scalar2=rstd,
            op0=OP.subtract, op1=OP.mult,
        )
        nc.vector.tensor_mul(o, o, gam)
        nc.vector.tensor_add(o, o, bet)
        nc.sync.dma_start(t["yout"][it * 128:(it + 1) * 128, :], o)

    for p_ in reversed(ctxs):
        p_.__exit__(None, None, None)


def _split_ctrl_waits(nc, maxw=1):
    """The container's walrus rejects instructions carrying more than 2 sem
    waits ("Too many sync wait commands"). Move excess waits onto preceding
    same-engine NoOps (engines execute their stream in order, so the waits
    still complete before the instruction issues)."""
    n = 0
    dma_types = (mybir.InstDMACopy, mybir.InstDMA)
    for bb in nc.main_func.blocks:
        changed = False
        out = []
        for ins in bb.instructions:
            lim = maxw
            si = ins.sync_info
            if si is not None and si.on_wait and len(si.on_wait) > lim:
                waits = list(si.on_wait)
                while len(waits) > lim:
                    chunk, waits = waits[:lim], waits[lim:]
                    nop = mybir.InstNoOp(
                        name=f"I-wsplit{n}",
                        engine=ins.engine,
                        sync_info=mybir.SyncInfo(on_wait=list(chunk), on_update=[]),
                    )
                    n += 1
                    out.append(nop)
                si.on_wait = waits
                changed = True
            out.append(ins)
        if changed:
            bb.instructions = out


@functools.lru_cache(maxsize=1)
def _build():
    nc = bass.Bass()
    t = {}
    t["hT"] = nc.dram_tensor("hT", [MODEL, TOT], F32, kind="ExternalInput")
    t["x_sm"] = nc.dram_tensor("x_sm", [SEG, MODEL], F32, kind="ExternalInput")
    t["rT"] = nc.dram_tensor("rT", [MODEL, TOT], F32, kind="ExternalInput")
    for w in ("wq", "wk", "wv", "wr"):
        t[w] = nc.dram_tensor(w, [MODEL, HD], F32, kind="ExternalInput")
    t["mlpw"] = nc.dram_tensor("mlpw", [HD, MODEL], BF16, kind="ExternalInput")
    t["u1l"] = nc.dram_tensor("u1l", [64, 16], BF16, kind="ExternalInput")
    t["u2l"] = nc.dram_tensor("u2l", [64, 16], BF16, kind="ExternalInput")
    t["maskc"] = nc.dram_tensor("maskc", [128, 4], F32, kind="ExternalInput")
    t["gam"] = nc.dram_tensor("gam", [1, MODEL], BF16, kind="ExternalInput")
    t["bet"] = nc.dram_tensor("bet", [1, MODEL], BF16, kind="ExternalInput")
    t["trim"] = nc.dram_tensor("trim", [128, 128], BF16, kind="ExternalInput")
    t["idm"] = nc.dram_tensor("idm", [128, 128], BF16, kind="ExternalInput")
    t["ebuf"] = nc.dram_tensor("ebuf", [SEG, H, TOT], BF16)
    t["qtd"] = nc.dram_tensor("qtd", [HD, SEG], BF16)
    t["ktd"] = nc.dram_tensor("ktd", [HD, TOT], BF16)
    t["vtd"] = nc.dram_tensor("vtd", [HD, TOT], BF16)
    t["rtd"] = nc.dram_tensor("rtd", [HD, TOT], BF16)
    t["attd"] = nc.dram_tensor("attd", [64, H * SEG], BF16)
    t["yout"] = nc.dram_tensor("yout", [SEG, MODEL], F32, kind="ExternalOutput")

    with tile.TileContext(nc) as tc:
        _emit(tc, t)
    _split_ctrl_waits(nc)
    return nc


def _host_inputs(inputs):
    x = np.asarray(inputs["x"], np.float32)
    mem = np.asarray(inputs["mem"], np.float32)
    att_mask = np.asarray(inputs["att_mask"], np.float32)
    u1 = np.asarray(inputs["u1"], np.float32).reshape(H, D)
    u2 = np.asarray(inputs["u2"], np.float32).reshape(H, D)
    R = np.asarray(inputs["R"], np.float32)

    h = np.concatenate([mem, x], axis=1)  # [B, TOT, MODEL]
    shared = {
        "rT": np.ascontiguousarray(R[-TOT:].T),
        "wq": np.asarray(inputs["w_q"], np.float32),
        "wk": np.asarray(inputs["w_k"], np.float32),
        "wv": np.asarray(inputs["w_v"], np.float32),
        "wr": np.asarray(inputs["w_r"], np.float32),
        "mlpw": np.asarray(inputs["mlp_w"], np.float32).astype(bf16_np),
        "u1l": np.ascontiguousarray(u1.T).astype(bf16_np),
        "u2l": np.ascontiguousarray(u2.T).astype(bf16_np),
        "gam": np.asarray(inputs["ln_gamma"], np.float32).reshape(1, MODEL).astype(bf16_np),
        "bet": np.asarray(inputs["ln_beta"], np.float32).reshape(1, MODEL).astype(bf16_np),
        "trim": np.tril(np.ones((128, 128), np.float32)).astype(bf16_np),
        "idm": np.eye(128, dtype=np.float32).astype(bf16_np),
    }
    in_maps = []
    for b in range(NCORES):
        m = dict(shared)
        m["hT"] = np.ascontiguousarray(h[b].T)
        m["x_sm"] = np.ascontiguousarray(x[b])
        m["maskc"] = np.ascontiguousarray(att_mask[b].reshape(4, 128).T)
        in_maps.append(m)
    return in_maps


def kernel(**inputs) -> np.ndarray:
    from concourse.bass_utils import run_bass_kernel_spmd

    nc = _build()
    in_maps = _host_inputs(inputs)
    res = run_bass_kernel_spmd(nc, in_maps, list(range(NCORES)))
    out = np.stack([np.asarray(res.results[b]["yout"]) for b in range(NCORES)])
    return out.astype(np.float32)



# revision 5
# speedup vs baseline: 14.4333x; 14.4333x over previous
"""Trainium2 Bass kernel for Transformer-XL style relative-position MHA.

Problem shapes (hardcoded): B=8, SEG=512, MEM=512, MODEL=1024, H=16, D=64.
Sharding: pure data-parallel over batch -> core b computes batch element b.

Head-split quirk: the reference splits heads with a FLAT reshape (torch
.view), so head h's key/value j lives at proj row 64h + j//16, columns
64*(j%16):+64 (32h + i//16 for queries).  Consequently, in the NATURAL
[T, HD] projection layout, head h's [T_h, 64] matrix is a CONTIGUOUS
row-major block of DRAM (rows 64h..64h+64 flattened).  We exploit that:

 - projections run in natural [t, hd] orientation (lhsT = hT tiles),
   staged to DRAM with the head-dim padded 64->128
   (qnat2/knat2/rnat2 [T*H? x 128]),
 - one whole-tensor XBAR DMA-transpose turns [16384, 128] into the
   per-head SBUF layout [128, h*TOT + j] directly (rows 64..127 junk),
 - V needs no transpose at all: vls[j, 64h+dd] chunks are contiguous.

u1/u2 are folded into qlay (qlay = q + u2 for the bd matmuls, then an
in-place DVE add of (u1-u2) turns it into q + u1 for the ac matmuls),
so scores are plain 64-row contractions.

Softmax is single-exp: raw bd goes through DRAM in diagonal coords
(ebuf row width 1152 per head, cols 1024..1152 preset to -1e9 so the
circulant-shift spill reads exp to 0 -> causal mask for free), the
skewed read lands bd_shift[i, j], a PE identity-matmul adds it onto the
ac PSUM accumulator, and one ACT exp with accum_out yields both p and
the row sums.
"""

import functools
import sys

import numpy as np

sys.path.insert(0, "/opt/trn_rl_repo")

import ml_dtypes  # noqa: E402

import concourse.bass as bass  # noqa: E402
import concourse.mybir as mybir  # noqa: E402
import concourse.tile as tile  # noqa: E402

B, SEG, MEM, MODEL, H, D = 8, 512, 512, 1024, 16, 64
TOT = SEG + MEM
HD = H * D
NCORES = 8
IT = SEG // 128                # 4 row tiles of 128 queries
JMAX = [640, 768, 896, 1024]   # per row-tile: columns beyond are fully masked
MMIN = [384, 256, 128, 0]      # per row-tile: smallest rel index m read
EW = TOT + 128                 # ebuf row width per head (128 cols of -1e9 pad)

F32 = mybir.dt.float32
BF16 = mybir.dt.bfloat16
AF = mybir.ActivationFunctionType
OP = mybir.AluOpType

bf16_np = ml_dtypes.bfloat16


def _emit(tc, t):
    nc = tc.nc
    ctxs = []

    def pool(name, bufs, space="SBUF"):
        p = tc.tile_pool(name=name, bufs=bufs, space=space)
        ctxs.append(p)
        return p.__enter__()

    csts = pool("csts", 1)
    htp = pool("htp", 8)          # hrT tiles [128, 2048] bf16
    wp = pool("wp", 8)            # weight stream [128, 1024] bf16
    stgp = pool("stgp", 2)        # projection staging [128, 2048] bf16
    layp = pool("layp", 1)        # rlay -> klay [128, 16384] bf16
    qlayp = pool("qlayp", 1)      # qlay [64, 8192] bf16
    vlsp = pool("vlsp", 1)        # qraw -> vls_all [128, 8192] bf16
    ebstp = pool("ebstp", 2)      # bd staging [128, 4*1152] bf16
    ebsp = pool("ebsp", 2)        # ebs quarter reads [128, 4096] bf16
    workp = pool("workp", 2)      # p tiles [128, 1024] bf16
    ptsp = pool("ptsp", 2)        # transposed p [128, 1024] bf16
    athp = pool("athp", 1)        # attTh [64, 8192] bf16
    midp = pool("midp", 1)        # attP rounds [64, 2048] bf16
    attsp = pool("attsp", 8)      # mlp lhsT [128, 512] bf16
    xyp = pool("xyp", 2)          # x / y / o tiles [128, 1024] f32
    small = pool("small", 4)
    ps = pool("ps", 2, space="PSUM")      # [128,1024] f32 (2 banks each)
    pst = pool("pst", 2, space="PSUM")    # transpose out [128, 512] bf16
    psv = pool("psv", 2, space="PSUM")    # pv accum [64, 128] f32

    # ---- constants ----
    ident = csts.tile([128, 128], BF16, tag="ident")
    nc.sync.dma_start(ident, t["idm"][:, :])
    u2l = csts.tile([64, 16], F32, tag="u2l")
    nc.sync.dma_start(u2l, t["u2l"][:, :])
    dul = csts.tile([64, 16], F32, tag="dul")
    nc.sync.dma_start(dul, t["dul"][:, :])
    masks = csts.tile([128, 4], F32, tag="masks")
    nc.sync.dma_start(masks, t["maskc"][:, :])
    epsb = csts.tile([128, 1], F32, tag="epsb")
    nc.vector.memset(epsb, 1e-5)
    gam = csts.tile([128, MODEL], BF16, tag="gam")
    nc.gpsimd.dma_start(gam, bass.AP(tensor=t["gam"], offset=0, ap=[[0, 128], [1, MODEL]]))
    bet = csts.tile([128, MODEL], BF16, tag="bet")
    nc.gpsimd.dma_start(bet, bass.AP(tensor=t["bet"], offset=0, ap=[[0, 128], [1, MODEL]]))

    # ---- load hrT = [hT | rT] tiles ----
    hts = []
    for mt in range(8):
        ht = htp.tile([128, 2 * TOT], BF16, tag="ht", name=f"ht{mt}")
        nc.sync.dma_start(ht, t["hrT"][mt * 128:(mt + 1) * 128, :])
        hts.append(ht)

    def stream_w(key):
        ws = []
        for mt in range(8):
            w = wp.tile([128, HD], BF16, tag="w", name=f"{key}{mt}")
            nc.sync.dma_start(w, t[key][mt * 128:(mt + 1) * 128, :])
            ws.append(w)
        return ws

    # ---- natural-layout projections ----
    # out rows t (partition), cols hd.  Staged to DRAM with head-dim padded
    # to 128 (q/k/r) or tight (v).
    def project(wkey, t_base, nt, dram, padded):
        ws = stream_w(wkey)
        for tt in range(nt):
            stg = stgp.tile([128, 2048], BF16, tag="stg", name=f"{wkey}st{tt}")
            so = stg[:, :].offset
            sstep = stg[:, :].ap[0][0]
            for half in range(2):
                acc = ps.tile([128, 1024], F32, tag="mm", name=f"{wkey}mm{tt}_{half}")
                for mt in range(8):
                    nc.tensor.matmul(
                        acc[:, 0:512],
                        lhsT=hts[mt][:, t_base + tt * 128:t_base + (tt + 1) * 128],
                        rhs=ws[mt][:, half * 512:(half + 1) * 512],
                        start=(mt == 0),
                        stop=(mt == 7),
                    )
                if padded:
                    dst = bass.AP(
                        tensor=stg[:, :].tensor,
                        offset=so + half * 1024,
                        ap=[[sstep, 128], [128, 8], [1, 64]],
                    )
                else:
                    dst = stg[:, half * 512:(half + 1) * 512]
                nc.scalar.copy(dst, acc[:, 0:512])
            width = 2048 if padded else 1024
            nc.gpsimd.dma_start(
                bass.AP(tensor=dram, offset=tt * 128 * width, ap=[[width, 128], [1, width]]),
                stg[:, :width],
            )

    project("wq", MEM, IT, t["qnat2"], True)   # x rows of h: t in [512, 1024)
    project("wr", TOT, 8, t["rnat2"], True)
    project("wv", 0, 8, t["vnat"], False)
    project("wk", 0, 8, t["knat2"], True)

    # ---- qlay: XBAR transpose + per-head u2 add ----
    qraw = vlsp.tile([128, 8192], BF16, tag="vls", name="qraw")
    nc.sync.dma_start(qraw, t["qnat2"][:, :], transpose=True)
    qlay = qlayp.tile([64, H * SEG], BF16, tag="qlay")
    for h in range(16):
        nc.vector.tensor_scalar(
            out=qlay[:, h * SEG:(h + 1) * SEG],
            in0=qraw[0:64, h * SEG:(h + 1) * SEG],
            scalar1=u2l[:, h:h + 1],
            scalar2=None,
            op0=OP.add,
        )

    # ---- rlay via XBAR, then bd production into ebuf (diagonal coords) ----
    rlay = layp.tile([128, H * TOT], BF16, tag="lay", name="rlay")
    nc.sync.dma_start(rlay, t["rnat2"][:, :], transpose=True)

    for it in range(IT):
        m0, w_ = MMIN[it], TOT - MMIN[it]
        i0 = it * 128
        for qq in range(4):
            ebq = ebstp.tile([128, 4 * EW], BF16, tag="ebq", name=f"ebq{it}_{qq}")
            eo = ebq[:, :].offset
            estep = ebq[:, :].ap[0][0]
            nc.vector.memset(
                bass.AP(tensor=ebq[:, :].tensor, offset=eo + TOT,
                        ap=[[estep, 128], [EW, 4], [1, EW - TOT]]),
                -1e9,
            )
            for hh in range(4):
                h = 4 * qq + hh
                acc = ps.tile([128, 1024], F32, tag="mm", name=f"bd{it}_{h}")
                c = m0
                while c < TOT:
                    cw = min(512, TOT - c)
                    nc.tensor.matmul(
                        acc[:, c - m0:c - m0 + cw],
                        lhsT=qlay[:, h * SEG + i0:h * SEG + i0 + 128],
                        rhs=rlay[0:64, h * TOT + c:h * TOT + c + cw],
                        start=True,
                        stop=True,
                    )
                    c += cw
                nc.scalar.copy(ebq[:, hh * EW + m0:hh * EW + TOT], acc[:, 0:w_])
            src = bass.AP(tensor=ebq[:, :].tensor, offset=eo + m0,
                          ap=[[estep, 128], [EW, 4], [1, EW - m0]])
            dst = bass.AP(tensor=t["ebuf"], offset=i0 * H * EW + qq * 4 * EW + m0,
                          ap=[[H * EW, 128], [EW, 4], [1, EW - m0]])
            nc.gpsimd.dma_start(dst, src)

    # ---- qlay becomes q + u1 (in-place), vls + klay loads ----
    for h in range(16):
        nc.vector.tensor_scalar(
            out=qlay[:, h * SEG:(h + 1) * SEG],
            in0=qlay[:, h * SEG:(h + 1) * SEG],
            scalar1=dul[:, h:h + 1],
            scalar2=None,
            op0=OP.add,
        )

    vls = vlsp.tile([128, 8192], BF16, tag="vls", name="vls")
    vo = vls[:, :].offset
    vstep = vls[:, :].ap[0][0]
    for h in range(16):
        src = bass.AP(tensor=t["vnat"], offset=h * TOT * 64,
                      ap=[[64, 128], [128 * 64, 8], [1, 64]])
        dst = bass.AP(tensor=vls[:, :].tensor, offset=vo + h * 64,
                      ap=[[vstep, 128], [1024, 8], [1, 64]])
        nc.sync.dma_start(dst, src)

    klay = layp.tile([128, H * TOT], BF16, tag="lay", name="klay")
    nc.sync.dma_start(klay, t["knat2"][:, :], transpose=True)

    # ---- scores / softmax / p@v ----
    attTh = athp.tile([64, H * SEG], BF16, tag="atth")
    for it in range(IT):
        jm = JMAX[it]
        nblk = jm // 128
        i0 = it * 128
        for qq in range(4):
            ebsq = ebsp.tile([128, 4096], BF16, tag="ebs", name=f"ebs{it}_{qq}")
            bo = ebsq[:, :].offset
            bstep = ebsq[:, :].ap[0][0]
            src = bass.AP(
                tensor=t["ebuf"],
                offset=i0 * H * EW + qq * 4 * EW + (511 - i0),
                ap=[[H * EW - 1, 128], [EW, 4], [1, jm]],
            )
            dst = bass.AP(tensor=ebsq[:, :].tensor, offset=bo,
                          ap=[[bstep, 128], [jm, 4], [1, jm]])
            nc.sync.dma_start(dst, src)
            for hh in range(4):
                h = 4 * qq + hh
                acc = ps.tile([128, 1024], F32, tag="mm", name=f"ac{it}_{h}")
                c = 0
                while c < jm:
                    cw = min(512, jm - c)
                    nc.tensor.matmul(
                        acc[:, c:c + cw],
                        lhsT=qlay[:, h * SEG + i0:h * SEG + i0 + 128],
                        rhs=klay[0:64, h * TOT + c:h * TOT + c + cw],
                        start=True,
                        stop=False,
                    )
                    nc.tensor.matmul(
                        acc[:, c:c + cw],
                        lhsT=ident,
                        rhs=ebsq[:, hh * jm + c:hh * jm + c + cw],
                        start=False,
                        stop=True,
                    )
                    c += cw
                p = workp.tile([128, 1024], BF16, tag="p", name=f"p{it}_{h}")
                sums = small.tile([128, 1], F32, tag="sums", name=f"sm{it}_{h}")
                nc.scalar.activation(p[:, :jm], acc[:, :jm], AF.Exp,
                                     scale=0.125, accum_out=sums)
                rec = small.tile([128, 1], F32, tag="rec", name=f"rc{it}_{h}")
                nc.vector.reciprocal(rec, sums)
                alpha = small.tile([128, 1], F32, tag="alpha", name=f"al{it}_{h}")
                nc.vector.tensor_mul(alpha, rec, masks[:, it:it + 1])
                nc.vector.tensor_scalar_mul(p[:, :jm], p[:, :jm], alpha)
                pts = ptsp.tile([128, 1024], BF16, tag="pts", name=f"pt{it}_{h}")
                for g in range(0, nblk, 4):
                    gn = min(4, nblk - g)
                    tp = pst.tile([128, 512], BF16, tag="tp", name=f"tp{it}_{h}_{g}")
                    for jj in range(gn):
                        nc.tensor.transpose(
                            tp[:, jj * 128:(jj + 1) * 128],
                            p[:, (g + jj) * 128:(g + jj + 1) * 128],
                            ident,
                        )
                    nc.vector.tensor_copy(
                        pts[:, g * 128:(g + gn) * 128], tp[:, 0:gn * 128]
                    )
                pv = psv.tile([64, 128], F32, tag="pv", name=f"pv{it}_{h}")
                for jb in range(nblk):
                    nc.tensor.matmul(
                        pv,
                        lhsT=vls[:, jb * 1024 + h * 64:jb * 1024 + h * 64 + 64],
                        rhs=pts[:, jb * 128:(jb + 1) * 128],
                        start=(jb == 0),
                        stop=(jb == nblk - 1),
                    )
                nc.scalar.copy(attTh[:, h * SEG + i0:h * SEG + i0 + 128], pv)

    # ---- att DRAM hop: attP[dd, cc*512+32h+rr] = attTh[dd, h*512+16rr+cc] ----
    ao = attTh[:, :].offset
    astep = attTh[:, :].ap[0][0]
    for rnd in range(4):
        attP = midp.tile([64, 2048], BF16, tag="mid", name=f"attP{rnd}")
        po = attP[:, :].offset
        pstep = attP[:, :].ap[0][0]
        src = bass.AP(
            tensor=attTh[:, :].tensor, offset=ao + 4 * rnd,
            ap=[[astep, 64], [1, 4], [512, 16], [16, 32]],
        )
        dst = bass.AP(
            tensor=attP[:, :].tensor, offset=po,
            ap=[[pstep, 64], [512, 4], [32, 16], [1, 32]],
        )
        nc.vector.tensor_copy(dst, src)
        nc.gpsimd.dma_start(
            bass.AP(tensor=t["attd"], offset=2048 * rnd,
                    ap=[[H * SEG, 64], [1, 2048]]),
            attP,
        )
    atts = []
    for a in range(8):
        at = attsp.tile([128, SEG], BF16, tag="att", name=f"att{a}")
        for ccp in range(2):
            src = bass.AP(
                tensor=t["attd"],
                offset=(2 * a + ccp) * 512,
                ap=[[H * SEG, 64], [1, 512]],
            )
            nc.sync.dma_start(at[ccp * 64:(ccp + 1) * 64, :], src)
        atts.append(at)

    # ---- mlp + residual + layernorm ----
    mlps = stream_w("mlpw")
    for it in range(IT):
        x = xyp.tile([128, MODEL], F32, tag="x", name=f"x{it}", bufs=1)
        nc.sync.dma_start(x, t["x_sm"][it * 128:(it + 1) * 128, :])
        acc = ps.tile([128, MODEL], F32, tag="mm", name=f"mlp{it}")
        for half in range(2):
            for dt in range(8):
                nc.tensor.matmul(
                    acc[:, half * 512:(half + 1) * 512],
                    lhsT=atts[dt][:, it * 128:(it + 1) * 128],
                    rhs=mlps[dt][:, half * 512:(half + 1) * 512],
                    start=(dt == 0),
                    stop=(dt == 7),
                )
        y = xyp.tile([128, MODEL], F32, tag="y", name=f"y{it}", bufs=1)
        ysum = small.tile([128, 1], F32, tag="ysum", name=f"ys{it}")
        nc.vector.scalar_tensor_tensor(
            out=y, in0=acc, scalar=1.0, in1=x,
            op0=OP.mult, op1=OP.add, accum_out=ysum,
        )
        sq = ps.tile([128, MODEL], F32, tag="mm", name=f"sq{it}")
        ysq = small.tile([128, 1], F32, tag="ysq", name=f"yq{it}")
        nc.scalar.activation(sq, y, AF.Square, accum_out=ysq)
        mu = small.tile([128, 1], F32, tag="mu", name=f"mu{it}")
        nc.scalar.mul(mu, ysum, 1.0 / MODEL)
        msq = small.tile([128, 1], F32, tag="msq", name=f"mq{it}")
        nc.scalar.mul(msq, ysq, 1.0 / MODEL)
        mu2 = small.tile([128, 1], F32, tag="mu2", name=f"m2{it}")
        nc.vector.tensor_mul(mu2, mu, mu)
        var = small.tile([128, 1], F32, tag="var", name=f"va{it}")
        nc.vector.tensor_tensor(out=var, in0=msq, in1=mu2, op=OP.subtract)
        std = small.tile([128, 1], F32, tag="std", name=f"sd{it}")
        nc.scalar.activation(std, var, AF.Sqrt, bias=epsb)
        rstd = small.tile([128, 1], F32, tag="rstd", name=f"rs{it}")
        nc.vector.reciprocal(rstd, std)
        o = xyp.tile([128, MODEL], F32, tag="o", name=f"o{it}", bufs=1)
        nc.vector.tensor_scalar(
            out=o, in0=y, scalar1=mu, scalar2=rstd,
            op0=OP.subtract, op1=OP.mult,
        )
        nc.vector.tensor_mul(o, o, gam)
        nc.vector.tensor_add(o, o, bet)
        nc.gpsimd.dma_start(t["yout"][it * 128:(it + 1) * 128, :], o)

    for p_ in reversed(ctxs):
        p_.__exit__(None, None, None)


def _split_ctrl_waits(nc, maxw=1):
    """The container's walrus rejects instructions carrying more than 2 sem
    waits ("Too many sync wait commands"). Move excess waits onto preceding
    same-engine NoOps (engines execute their stream in order, so the waits
    still complete before the instruction issues)."""
    n = 0
    for bb in nc.main_func.blocks:
        changed = False
        out = []
        for ins in bb.instructions:
            lim = maxw
            si = ins.sync_info
            if si is not None and si.on_wait and len(si.on_wait) > lim:
                waits = list(si.on_wait)
                while len(waits) > lim:
                    chunk, waits = waits[:lim], waits[lim:]
                    nop = mybir.InstNoOp(
                        name=f"I-wsplit{n}",
                        engine=ins.engine,
                        sync_info=mybir.SyncInfo(on_wait=list(chunk), on_update=[]),
                    )
                    n += 1
                    out.append(nop)
                si.on_wait = waits
                changed = True
            out.append(ins)
        if changed:
            bb.instructions = out


@functools.lru_cache(maxsize=1)
def _build():
    nc = bass.Bass()
    t = {}
    t["hrT"] = nc.dram_tensor("hrT", [MODEL, 2 * TOT], BF16, kind="ExternalInput")
    t["x_sm"] = nc.dram_tensor("x_sm", [SEG, MODEL], F32, kind="ExternalInput")
    for w in ("wq", "wk", "wv", "wr"):
        t[w] = nc.dram_tensor(w, [MODEL, HD], BF16, kind="ExternalInput")
    t["mlpw"] = nc.dram_tensor("mlpw", [HD, MODEL], BF16, kind="ExternalInput")
    t["u2l"] = nc.dram_tensor("u2l", [64, 16], F32, kind="ExternalInput")
    t["dul"] = nc.dram_tensor("dul", [64, 16], F32, kind="ExternalInput")
    t["maskc"] = nc.dram_tensor("maskc", [128, 4], F32, kind="ExternalInput")
    t["gam"] = nc.dram_tensor("gam", [1, MODEL], BF16, kind="ExternalInput")
    t["bet"] = nc.dram_tensor("bet", [1, MODEL], BF16, kind="ExternalInput")
    t["idm"] = nc.dram_tensor("idm", [128, 128], BF16, kind="ExternalInput")
    t["qnat2"] = nc.dram_tensor("qnat2", [SEG * H, 128], BF16)
    t["knat2"] = nc.dram_tensor("knat2", [TOT * H, 128], BF16)
    t["rnat2"] = nc.dram_tensor("rnat2", [TOT * H, 128], BF16)
    t["vnat"] = nc.dram_tensor("vnat", [TOT * H, 64], BF16)
    t["ebuf"] = nc.dram_tensor("ebuf", [SEG, H * EW], BF16)
    t["attd"] = nc.dram_tensor("attd", [64, H * SEG], BF16)
    t["yout"] = nc.dram_tensor("yout", [SEG, MODEL], F32, kind="ExternalOutput")

    with tile.TileContext(nc) as tc:
        _emit(tc, t)
    _split_ctrl_waits(nc)
    return nc


def _host_inputs(inputs):
    x = np.asarray(inputs["x"], np.float32)
    mem = np.asarray(inputs["mem"], np.float32)
    att_mask = np.asarray(inputs["att_mask"], np.float32)
    u1 = np.asarray(inputs["u1"], np.float32).reshape(H, D)
    u2 = np.asarray(inputs["u2"], np.float32).reshape(H, D)
    R = np.asarray(inputs["R"], np.float32)

    h = np.concatenate([mem, x], axis=1)  # [B, TOT, MODEL]
    rT = np.ascontiguousarray(R[-TOT:].T)
    shared = {
        "wq": np.asarray(inputs["w_q"], np.float32).astype(bf16_np),
        "wk": np.asarray(inputs["w_k"], np.float32).astype(bf16_np),
        "wv": np.asarray(inputs["w_v"], np.float32).astype(bf16_np),
        "wr": np.asarray(inputs["w_r"], np.float32).astype(bf16_np),
        "mlpw": np.asarray(inputs["mlp_w"], np.float32).astype(bf16_np),
        "u2l": np.ascontiguousarray(u2.T),
        "dul": np.ascontiguousarray(u1 - u2).T.copy(),
        "gam": np.asarray(inputs["ln_gamma"], np.float32).reshape(1, MODEL).astype(bf16_np),
        "bet": np.asarray(inputs["ln_beta"], np.float32).reshape(1, MODEL).astype(bf16_np),
        "idm": np.eye(128, dtype=np.float32).astype(bf16_np),
    }
    in_maps = []
    for b in range(NCORES):
        m = dict(shared)
        m["hrT"] = np.concatenate([h[b].T, rT], axis=1).astype(bf16_np)
        m["x_sm"] = np.ascontiguousarray(x[b])
        m["maskc"] = np.ascontiguousarray(att_mask[b].reshape(4, 128).T)
        in_maps.append(m)
    return in_maps


def kernel(**inputs) -> np.ndarray:
    from concourse.bass_utils import run_bass_kernel_spmd

    nc = _build()
    in_maps = _host_inputs(inputs)
    res = run_bass_kernel_spmd(nc, in_maps, list(range(NCORES)))
    out = np.stack([np.asarray(res.results[b]["yout"]) for b in range(NCORES)])
    return out.astype(np.float32)
